# revision 1
# baseline (speedup 1.0000x reference)
"""TRN2 Bass kernel for nn_GCNEModel (3-layer GCN + dense head), 8 NeuronCores.

Sharding: data-parallel over the batch axis — each core runs one sample's
full GCN. The scatter-add aggregation is restructured as pure row-gathers:
nodes are relabeled by descending in-degree (the relabeling is absorbed into
host-side permutations of pel_W / x / lin1_W), so "round k" (the k-th
incoming edge of every node that has one) targets a contiguous node prefix.
Per layer, per core:

  t = x @ W^T            PE GEMM (stationary = feat-major input chunk)
  m = dinv * t           DVE, node-major [128, 119, 64]; DMA'd to HBM rows
  s = m + sum_k m[src_k] MoE-style dma_gather rounds + DVE adds
  h = relu(dinv*s^T + b) DVE mult, PE transpose, ACT bias+relu (feat-major)
  g += h^T @ w_l         PE matvec chunks (fc folded per layer)

head: z = relu(sum_j g_j^T @ lin1_W^T_j + b1); host: lin2 + log_softmax.
"""
import os
import sys

os.environ.setdefault("NEURON_RT_RESET_CORES", "1")
for _p in ("/opt/trn_rl_repo", "/root/.axon_site/_ro/trn_rl_repo"):
    if os.path.isdir(_p) and _p not in sys.path:
        sys.path.insert(0, _p)

from dataclasses import dataclass, field

import numpy as np

import concourse.bacc as bacc
import concourse.mybir as mybir
import concourse.tile as tile
from concourse.bass_utils import run_bass_kernel_spmd

P = 128
HID = 64
FIN = 36
NFC = 256
N_CORES = 8


@dataclass
class Cfg:
    n: int
    chunk: int = 3968   # gather chunk; <= ~4080 (SWDGE ring: 256 descs/engine)
    npad: int = field(init=False)
    nslot: int = field(init=False)

    def __post_init__(self):
        self.nslot = (self.n + P - 1) // P
        self.npad = self.nslot * P


def preprocess(cfg: Cfg, edge_index: np.ndarray):
    n, npad, nslot = cfg.n, cfg.npad, cfg.nslot
    src_old = np.asarray(edge_index[0], dtype=np.int64)
    dst_old = np.asarray(edge_index[1], dtype=np.int64)
    E = src_old.shape[0]

    deg = np.bincount(dst_old, minlength=n)
    pi = np.argsort(-deg, kind="stable")
    inv_pi = np.empty(n, dtype=np.int64)
    inv_pi[pi] = np.arange(n)
    deg_s = deg[pi]

    dinv_pad = np.zeros(npad)
    dinv_pad[:n] = 1.0 / np.sqrt(deg_s.astype(np.float64) + 1.0)

    src_new = inv_pi[src_old]
    dst_new = inv_pi[dst_old]

    order = np.argsort(dst_new, kind="stable")
    src_sorted = src_new[order]
    dst_sorted = dst_new[order]
    starts = np.zeros(n + 1, dtype=np.int64)
    np.cumsum(np.bincount(dst_new, minlength=n), out=starts[1:])
    kpos = np.arange(E) - starts[dst_sorted]

    Kmax = int(deg_s[0]) if E else 0
    DUMMY_SIG = n
    assert npad > n

    def r_of(sig):
        return (sig % P) * nslot + sig // P

    idx_stream = []
    segments = []
    stream_blk = 0
    for k in range(Kmax):
        n_k = int(np.searchsorted(-deg_s, -(k + 1), side="right"))
        sel = kpos == k
        srcs_k = src_sorted[sel][np.argsort(dst_sorted[sel], kind="stable")]
        assert srcs_k.shape[0] == n_k
        nblk = (n_k + P - 1) // P
        padded = np.full(nblk * P, DUMMY_SIG, dtype=np.int64)
        padded[:n_k] = srcs_k
        idx_stream.append(r_of(padded))
        segments.append((stream_blk, nblk))
        stream_blk += nblk

    idx_stream = (np.concatenate(idx_stream) if idx_stream
                  else np.zeros(0, np.int64))
    Eprime = idx_stream.shape[0]

    chunks = []
    pos = 0
    while pos < Eprime:
        c = min(cfg.chunk, Eprime - pos)
        chunks.append((pos, c))
        pos += c

    chunk_adds = []
    for (cstart, clen) in chunks:
        c_b0, c_b1 = cstart // P, (cstart + clen) // P
        adds = []
        for (sb, nb) in segments:
            lo, hi = max(c_b0, sb), min(c_b1, sb + nb)
            if lo < hi:
                adds.append((lo - c_b0, lo - sb, hi - lo))
        chunk_adds.append(adds)

    cols = Eprime // 16
    arr16 = idx_stream.reshape(cols, 16).T.astype(np.int16)
    idx_wrapped = np.ascontiguousarray(np.tile(arr16, (8, 1)))

    return dict(pi=pi, deg_s=deg_s, dinv_pad=dinv_pad,
                idx_stream=idx_stream, idx_wrapped=idx_wrapped,
                chunks=chunks, chunk_adds=chunk_adds, Eprime=Eprime)


def build_constants(cfg: Cfg, prep, inputs):
    n, npad, nslot = cfg.n, cfg.npad, cfg.nslot
    pi = prep["pi"]
    dinv_pad = prep["dinv_pad"].astype(np.float32)

    pel_W = np.asarray(inputs["pel_W"], np.float32)
    pel_b = np.asarray(inputs["pel_b"], np.float32)
    pe_perm = (pel_W.T + pel_b)[pi]

    x = np.asarray(inputs["x"], np.float32)
    bs = x.shape[0]
    x_fm = np.zeros((bs, FIN, npad), np.float32)
    for s in range(bs):
        xc = np.concatenate([x[s][pi], pe_perm], axis=1)
        x_fm[s, :, :n] = xc.T

    def to_node_major(v):
        return np.ascontiguousarray(v.reshape(nslot, P).T)

    dinv_nm = to_node_major(dinv_pad)
    dinv64 = np.ascontiguousarray(
        np.repeat(dinv_nm[:, :, None], HID, axis=2)).reshape(P, nslot * HID)
    mask = np.zeros(npad, np.float32)
    mask[:n] = 1.0
    mask_nm = to_node_major(mask)

    Wc = [np.ascontiguousarray(np.asarray(inputs[f"conv{i}_W"], np.float32).T)
          for i in (1, 2, 3)]
    bc = [np.ascontiguousarray(np.asarray(inputs[f"conv{i}_b"], np.float32)
                               .reshape(HID, 1)) for i in (1, 2, 3)]

    fc_W = np.asarray(inputs["fc_W"], np.float32).reshape(-1)
    w_l = [np.ascontiguousarray(fc_W[l::3].reshape(HID, 1)) for l in range(3)]
    fc_b = float(np.asarray(inputs["fc_b"], np.float32).reshape(()))

    lin1_W = np.asarray(inputs["lin1_W"], np.float32)
    W1T = np.zeros((npad, NFC), np.float32)
    W1T[:n] = lin1_W[:, pi].T
    b1_eff = np.ascontiguousarray(
        (np.asarray(inputs["lin1_b"], np.float32)
         + fc_b * lin1_W.sum(axis=1)).reshape(1, NFC))

    return dict(x_fm=x_fm, dinv64=dinv64, mask_nm=mask_nm, Wc=Wc, bc=bc,
                w_l=w_l, W1T=W1T, b1_eff=b1_eff)


def build_program(cfg: Cfg, prep, gb_bufs=2, idx_bufs=4, xw_bufs=2, w1_bufs=4, pt_bufs=3, ptr_bufs=2, gemm_grp=8, tr_grp=4):
    n, npad, nslot = cfg.n, cfg.npad, cfg.nslot
    chunks, chunk_adds = prep["chunks"], prep["chunk_adds"]
    cols_total = prep["Eprime"] // 16
    f32 = mybir.dt.float32

    nc = bacc.Bacc("TRN2", debug=False)

    x_dram = nc.dram_tensor("x_fm", [FIN, npad], f32, kind="ExternalInput")
    dinv_dram = nc.dram_tensor("dinv64", [P, nslot * HID], f32, kind="ExternalInput")
    mask_dram = nc.dram_tensor("mask_nm", [P, nslot], f32, kind="ExternalInput")
    Wc_dram = [nc.dram_tensor(f"Wc{i}", [FIN if i == 0 else HID, HID], f32,
                              kind="ExternalInput") for i in range(3)]
    bc_dram = [nc.dram_tensor(f"bc{i}", [HID, 1], f32, kind="ExternalInput")
               for i in range(3)]
    wl_dram = [nc.dram_tensor(f"wl{i}", [HID, 1], f32, kind="ExternalInput")
               for i in range(3)]
    idx_dram = nc.dram_tensor("idx_all", [P, cols_total], mybir.dt.int16,
                              kind="ExternalInput")
    w1t_dram = nc.dram_tensor("W1T", [npad, NFC], f32, kind="ExternalInput")
    b1_dram = nc.dram_tensor("b1_eff", [1, NFC], f32, kind="ExternalInput")
    ident_dram = nc.dram_tensor("ident", [P, P], f32, kind="ExternalInput")
    z_dram = nc.dram_tensor("z", [1, NFC], f32, kind="ExternalOutput")

    m_dram = [nc.dram_tensor(f"m_hbm{i}", [npad, HID], f32) for i in range(3)]

    with tile.TileContext(nc) as tc:
        with (
            tc.tile_pool(name="const", bufs=1) as cpool,
            tc.tile_pool(name="state", bufs=1) as spool,
            tc.tile_pool(name="xw", bufs=xw_bufs) as xpool,
            tc.tile_pool(name="idx", bufs=idx_bufs) as ipool,
            tc.tile_pool(name="gath", bufs=gb_bufs) as gpool,
            tc.tile_pool(name="w1t", bufs=w1_bufs) as wpool,
            tc.tile_pool(name="psum_t", bufs=pt_bufs, space="PSUM") as pt_pool,
            tc.tile_pool(name="psum_tr", bufs=ptr_bufs, space="PSUM") as ptr_pool,
            tc.tile_pool(name="psum_g", bufs=1, space="PSUM") as pg_pool,
            tc.tile_pool(name="psum_z", bufs=1, space="PSUM") as pz_pool,
        ):
            dinv64 = cpool.tile([P, nslot, HID], f32, tag="dinv64")
            nc.sync.dma_start(out=dinv64[:], in_=dinv_dram[:].rearrange(
                "p (g f) -> p g f", f=HID))
            mask_sb = cpool.tile([P, nslot], f32, tag="mask")
            nc.sync.dma_start(out=mask_sb[:], in_=mask_dram[:])
            ident = cpool.tile([P, P], f32, tag="ident")
            nc.sync.dma_start(out=ident[:], in_=ident_dram[:])
            Wc_sb, bc_sb, wl_sb = [], [], []
            for i in range(3):
                w = cpool.tile([FIN if i == 0 else HID, HID], f32, tag=f"Wc{i}")
                nc.sync.dma_start(out=w[:], in_=Wc_dram[i][:])
                Wc_sb.append(w)
                b = cpool.tile([HID, 1], f32, tag=f"bc{i}")
                nc.sync.dma_start(out=b[:], in_=bc_dram[i][:])
                bc_sb.append(b)
                wl = cpool.tile([HID, 1], f32, tag=f"wl{i}")
                nc.sync.dma_start(out=wl[:], in_=wl_dram[i][:])
                wl_sb.append(wl)
            b1_sb = cpool.tile([1, NFC], f32, tag="b1")
            nc.sync.dma_start(out=b1_sb[:], in_=b1_dram[:])

            g_acc = spool.tile([P, nslot], f32, tag="g_acc")
            nc.vector.memset(g_acc[:], 0.0)
            m_sb = spool.tile([P, nslot, HID], f32, tag="m")
            s_sb = spool.tile([P, nslot, HID], f32, tag="s")
            h_fm = spool.tile([HID, npad], f32, tag="h_fm")

            GEMM_GRP = gemm_grp
            TR_GRP = tr_grp
            HGRP = 4
            psum_z = pz_pool.tile([1, NFC], f32, tag="pz")
            head_state = dict(emitted=0, total=nslot)

            def emit_head_seg(si, b0, b1):
                # finalize g for this segment and fold it into the lin1
                # matvec while layer-3 gathers still run
                nc.vector.tensor_add(g_acc[:, b0:b1], g_acc[:, b0:b1],
                                     psum_g[:, b0:b1])
                nc.vector.tensor_mul(g_acc[:, b0:b1], g_acc[:, b0:b1],
                                     mask_sb[:, b0:b1])
                for g0 in range(b0, b1, HGRP):
                    gn = min(HGRP, b1 - g0)
                    w1t = wpool.tile([P, HGRP, NFC], f32, tag="w1t")
                    nc.sync.dma_start(
                        out=w1t[:, :gn, :],
                        in_=w1t_dram[:].rearrange(
                            "(g p) f -> p g f", p=P)[:, g0:g0 + gn, :])
                    for j in range(gn):
                        jj = g0 + j
                        nc.tensor.matmul(
                            psum_z[:], g_acc[:, jj:jj + 1], w1t[:, j, :],
                            start=(head_state["emitted"] == 0),
                            stop=(head_state["emitted"] == head_state["total"] - 1))
                        head_state["emitted"] += 1

            for l in range(3):
                for g0 in range(0, nslot, GEMM_GRP):
                    gn = min(GEMM_GRP, nslot - g0)
                    psum_t = pt_pool.tile([P, GEMM_GRP, HID], f32, tag="pt")
                    if l == 0:
                        xt = xpool.tile([FIN, GEMM_GRP * P], f32, tag="xt")
                        nc.sync.dma_start(
                            out=xt[:, :gn * P],
                            in_=x_dram[:, g0 * P:(g0 + gn) * P])
                    for j in range(gn):
                        if l == 0:
                            lhsT = xt[:, j * P:(j + 1) * P]
                        else:
                            lhsT = h_fm[:, (g0 + j) * P:(g0 + j + 1) * P]
                        nc.tensor.matmul(psum_t[:, j, :], lhsT, Wc_sb[l][:],
                                         start=True, stop=True)
                    nc.vector.tensor_mul(m_sb[:, g0:g0 + gn, :],
                                         psum_t[:, :gn, :],
                                         dinv64[:, g0:g0 + gn, :])
                    nc.sync.dma_start(
                        out=m_dram[l][:].rearrange("(p g) f -> p g f",
                                                   p=P)[:, g0:g0 + gn, :],
                        in_=m_sb[:, g0:g0 + gn, :])
                nc.vector.tensor_copy(s_sb[:], m_sb[:])
                # segment the post-gather tail: segment [b0,b1) of node blocks
                # is complete after the last gather chunk whose adds touch it;
                # emit its mult/transpose/relu/matvec right there so it
                # overlaps the remaining gather stream.
                seg_bounds = []
                b0 = 0
                for sz in ([4] * 4 + [8] * 2 + [16] * 16):
                    if b0 >= nslot:
                        break
                    seg_bounds.append((b0, min(b0 + sz, nslot)))
                    b0 += sz
                segs = []
                for (b0, b1) in seg_bounds:
                    last = 0
                    for ci, adds in enumerate(chunk_adds):
                        if any(sb < b1 and sb + nb > b0 for (_, sb, nb) in adds):
                            last = ci
                    segs.append((b0, b1, last))
                psum_g = pg_pool.tile([P, nslot], f32, tag="pg")

                def emit_seg_tail(b0, b1):
                    nc.vector.tensor_mul(s_sb[:, b0:b1, :],
                                         s_sb[:, b0:b1, :],
                                         dinv64[:, b0:b1, :])
                    for g0 in range(b0, b1, TR_GRP):
                        gn = min(TR_GRP, b1 - g0)
                        psum_tr = ptr_pool.tile([HID, TR_GRP, P], f32,
                                                tag="ptr")
                        for j in range(gn):
                            nc.tensor.transpose(psum_tr[:, j, :],
                                                s_sb[:, g0 + j, :], ident[:])
                        nc.scalar.activation(
                            h_fm[:, g0 * P:(g0 + gn) * P],
                            psum_tr[:, :gn, :].rearrange("f g p -> f (g p)"),
                            mybir.ActivationFunctionType.Relu,
                            bias=bc_sb[l][:], scale=1.0)
                    for j in range(b0, b1):
                        nc.tensor.matmul(psum_g[:, j:j + 1],
                                         h_fm[:, j * P:(j + 1) * P],
                                         wl_sb[l][:], start=True, stop=True)

                for ci, (cstart, clen) in enumerate(chunks):
                    cblk = clen // P
                    idx_t = ipool.tile([P, cfg.chunk // 16], mybir.dt.int16,
                                       tag="idx")
                    nc.sync.dma_start(
                        out=idx_t[:, :clen // 16],
                        in_=idx_dram[:, cstart // 16:(cstart + clen) // 16])
                    gbuf = gpool.tile([P, (cfg.chunk + P - 1) // P, HID], f32,
                                      tag="gb")
                    nc.gpsimd.dma_gather(
                        gbuf[:, :cblk, :], m_dram[l][:], idx_t[:, :clen // 16],
                        clen, clen, HID, single_packet=False)
                    for (gb, sb, nb) in chunk_adds[ci]:
                        nc.vector.tensor_add(s_sb[:, sb:sb + nb, :],
                                             s_sb[:, sb:sb + nb, :],
                                             gbuf[:, gb:gb + nb, :])
                    for si, (b0, b1, last) in enumerate(segs):
                        if last == ci:
                            emit_seg_tail(b0, b1)
                            if l == 2:
                                emit_head_seg(si, b0, b1)
                if l < 2:
                    nc.vector.tensor_add(g_acc[:], g_acc[:], psum_g[:])

            assert head_state["emitted"] == nslot, head_state
            z_sb = spool.tile([1, NFC], f32, tag="z")
            nc.vector.tensor_add(z_sb[:], psum_z[:], b1_sb[:])
            nc.vector.tensor_relu(z_sb[:], z_sb[:])
            nc.sync.dma_start(out=z_dram[:], in_=z_sb[:])

    nc.compile()
    return nc


def make_in_maps(cfg: Cfg, prep, consts, n_cores=N_CORES):
    eye = np.eye(P, dtype=np.float32)
    shared = dict(
        dinv64=consts["dinv64"], mask_nm=consts["mask_nm"],
        idx_all=prep["idx_wrapped"], W1T=consts["W1T"],
        b1_eff=consts["b1_eff"], ident=eye,
    )
    for i in range(3):
        shared[f"Wc{i}"] = consts["Wc"][i]
        shared[f"bc{i}"] = consts["bc"][i]
        shared[f"wl{i}"] = consts["w_l"][i]
    return [dict(shared, x_fm=np.ascontiguousarray(consts["x_fm"][c]))
            for c in range(n_cores)]


def finish_host(z_all, inputs):
    W2 = np.asarray(inputs["lin2_W"], np.float32)
    b2 = np.asarray(inputs["lin2_b"], np.float32)
    logits = z_all @ W2.T + b2
    mx = logits.max(axis=1, keepdims=True)
    e = np.exp(logits - mx)
    return ((logits - mx) - np.log(e.sum(axis=1, keepdims=True))).astype(np.float32)


_PROGRAM_CACHE = {}


def _get_program(cfg: Cfg, prep, cache_key):
    hit = _PROGRAM_CACHE.get(cache_key)
    if hit is None:
        hit = build_program(cfg, prep)
        _PROGRAM_CACHE[cache_key] = hit
    return hit


def _reset_device():
    """Run a trivial program to clear a wedged exec unit (observed to help)."""
    try:
        nc = bacc.Bacc("TRN2", debug=False)
        a = nc.dram_tensor("a", [P, 64], mybir.dt.float32, kind="ExternalInput")
        b = nc.dram_tensor("b", [P, 64], mybir.dt.float32, kind="ExternalOutput")
        with tile.TileContext(nc) as tc:
            with tc.tile_pool(name="p", bufs=1) as pool:
                t = pool.tile([P, 64], mybir.dt.float32)
                nc.sync.dma_start(out=t[:], in_=a[:])
                nc.sync.dma_start(out=b[:], in_=t[:])
        nc.compile()
        run_bass_kernel_spmd(
            nc, [{"a": np.zeros((P, 64), np.float32)}] * N_CORES,
            list(range(N_CORES)))
    except Exception:
        pass


def kernel(**inputs) -> np.ndarray:
    x = np.asarray(inputs["x"])
    bs, n = x.shape[0], x.shape[1]
    assert bs == N_CORES, f"expected batch {N_CORES}, got {bs}"

    cfg = Cfg(n=n)
    edge_index = np.asarray(inputs["edge_index"])
    prep = preprocess(cfg, edge_index)
    cache_key = (n, edge_index.shape[1],
                 hash(edge_index.tobytes()))
    nc = _get_program(cfg, prep, cache_key)
    consts = build_constants(cfg, prep, inputs)
    in_maps = make_in_maps(cfg, prep, consts)

    last_err = None
    for attempt in range(3):
        try:
            res = run_bass_kernel_spmd(nc, in_maps, list(range(N_CORES)))
            break
        except Exception as e:  # wedged device — reset and retry
            last_err = e
            _reset_device()
    else:
        raise last_err

    z_all = np.stack([res.results[c]["z"][0] for c in range(N_CORES)])
    return finish_host(z_all, inputs)



# revision 16
# speedup vs baseline: 1.3177x; 1.3177x over previous
"""TRN2 Bass kernel for nn_GCNEModel (3-layer GCN + dense head), 8 NeuronCores.

Sharding: data-parallel over the batch axis — each core runs one sample's
full GCN. The scatter-add aggregation runs on the DMA engines' CCE add path
(dma_scatter_add): per layer, m = dinv*(x@W^T) is held in SBUF node-major
fp16; "round k" scatter-adds each node's k-th out-edge message directly into
HBM s rows (128B fp16 payloads — half the bytes/descriptor of the fp32
gather formulation, and no DVE adds). Rounds are built host-side so that
within any single scatter op all real destinations are distinct (the CCE
read-modify-write path loses updates on same-row races inside one op); two
ping-pong column-halves of the 256B s rows let consecutive ops overlap on
the DMA engines while same-half ops serialize. The self-loop term doubles
as the row init: s_row(A) = m, s_row(B) = 0, overwritten each layer.

Per layer, per core:
  t = x @ W^T               PE fp16 GEMM (psum f32)
  m = dinv * t -> fp16      DVE, node-major [128, 119, 64]
  s_hbm[:,A] = m; [:,B] = 0 strided 128B-elem DMA init (self-loop term)
  s_hbm[dst] += m[src]      dma_scatter_add rounds (dst-distinct per op)
  s = (A+B) * dinv          readback + DVE
  h = relu(s^T + b)         PE transpose, ACT bias+relu (feature-major fp16)
  g += h^T @ w_l            PE matvec into persistent psum (fc folded)

head: z = relu(g^T @ lin1_W^T + b1); host: lin2 + log_softmax.
"""
import os
import sys

os.environ.setdefault("NEURON_RT_RESET_CORES", "1")
for _p in ("/opt/trn_rl_repo", "/root/.axon_site/_ro/trn_rl_repo"):
    if os.path.isdir(_p) and _p not in sys.path:
        sys.path.insert(0, _p)

from dataclasses import dataclass, field

import numpy as np

import concourse.bacc as bacc
import concourse.mybir as mybir
import concourse.tile as tile
from concourse.bass_utils import run_bass_kernel_spmd

P = 128
HID = 64
FIN = 36
NFC = 256
N_CORES = 8
CHUNK_SLOTS = 30          # nodes per scatter op = 30*128 = 3840 (SWDGE ring cap ~4080)


@dataclass
class Cfg:
    n: int
    nslot: int = field(init=False)
    npad: int = field(init=False)
    srows: int = field(init=False)   # s_hbm rows: nodes + 1 dummy slot/partition
    sslot: int = field(init=False)

    def __post_init__(self):
        self.nslot = (self.n + P - 1) // P
        self.npad = self.nslot * P
        self.sslot = self.nslot + 1
        self.srows = self.sslot * P


def preprocess(cfg: Cfg, edge_index: np.ndarray):
    """Relabel nodes by out-degree desc; build scatter rounds with per-op
    distinct real dsts (2 ping-pong buffers; conflicting edges defer)."""
    n, nslot, sslot = cfg.n, cfg.nslot, cfg.sslot
    src_old = np.asarray(edge_index[0], dtype=np.int64)
    dst_old = np.asarray(edge_index[1], dtype=np.int64)

    outdeg = np.bincount(src_old, minlength=n)
    pi = np.argsort(-outdeg, kind="stable")          # new label -> old node
    inv_pi = np.empty(n, dtype=np.int64)
    inv_pi[pi] = np.arange(n)

    indeg = np.bincount(dst_old, minlength=n)
    dinv_pad = np.zeros(cfg.npad)
    dinv_pad[:n] = 1.0 / np.sqrt(indeg[pi].astype(np.float64) + 1.0)

    s_new = inv_pi[src_old]
    d_new = inv_pi[dst_old]

    def r_of(node):
        return (node % P) * sslot + node // P        # s_hbm row of a node

    # per-node dst lists (in new labels), nodes in degree-desc position order
    eorder = np.argsort(s_new, kind="stable")
    ss, dd = s_new[eorder], d_new[eorder]
    starts = np.zeros(n + 1, np.int64)
    np.cumsum(np.bincount(ss, minlength=n), out=starts[1:])
    deg = (starts[1:] - starts[:-1]).copy()
    Kmax = int(deg[0]) if n else 0
    CH = CHUNK_SLOTS * P

    # Per-node matching: assign node i's deg edges to rounds {0..deg-1} so
    # that within each (round, positional chunk) all dsts are distinct
    # (one scatter op = one chunk of one round = one of 2 ping-pong column
    # halves; same-half ops serialize, so only within-op dups race).
    KCAP = Kmax + 8
    used = [[set() for _ in range((n + CH - 1) // CH)] for _ in range(KCAP)]
    round_dst = np.full((KCAP, n), -1, np.int64)   # [k, i] -> dst or -1
    overflow = 0
    for i in range(n):
        d = int(deg[i])
        if d == 0:
            continue
        edges = dd[starts[i]:starts[i + 1]]
        c = i // CH
        nr = d          # rounds available to this node; grows on overflow
        while True:
            match_er = [-1] * d          # edge j -> round
            round_e = [-1] * nr          # round k -> edge

            def try_assign(jj, seen):
                ee = int(edges[jj])
                for k in range(nr):
                    if seen[k] or ee in used[k][c]:
                        continue
                    seen[k] = True
                    if round_e[k] == -1 or try_assign(round_e[k], seen):
                        round_e[k] = jj
                        match_er[jj] = k
                        return True
                return False

            ok = all(try_assign(j, [False] * nr) for j in range(d))
            if ok:
                break
            nr += 2
            overflow += 1
            assert nr <= KCAP, "matching diverged"
        for j in range(d):
            k = match_er[j]
            e = int(edges[j])
            used[k][c].add(e)
            round_dst[k, i] = e

    n_k = []
    for k in range(KCAP):
        nz = np.nonzero(round_dst[k] >= 0)[0]
        n_k.append(int(nz[-1]) + 1 if nz.size else 0)
    Kmax = max(k for k in range(KCAP) if n_k[k] > 0) + 1
    n_k = n_k[:Kmax]

    ops = []          # (c0_slot, ns, num_idxs, nval, buf, idx_off, idx_cols)
    idx_chunks = []
    idx_cols_total = 0
    holes = 0
    dummy_ctr = 0
    for k in range(Kmax):
        for ci, c0 in enumerate(range(0, n_k[k], CH)):
            end = min(c0 + CH, n_k[k])
            buf = (k + ci) % 2
            seg = round_dst[k, c0:end].copy()
            nval = int(np.nonzero(seg >= 0)[0][-1]) + 1
            num_idxs = (nval + 15) // 16 * 16
            padded = np.full(num_idxs, -1, np.int64)
            real = seg[:nval] >= 0
            padded[:nval][real] = r_of(seg[:nval][real])
            nholes = int((~real).sum())
            if nholes:
                hh = np.nonzero(~real)[0]
                padded[hh] = ((dummy_ctr + np.arange(nholes)) * 37 % P) \
                    * sslot + (sslot - 1)
                dummy_ctr += nholes
                holes += nholes
            assert (padded[:nval] >= 0).all()
            cols = num_idxs // 16
            idx_chunks.append(
                np.ascontiguousarray(padded.reshape(cols, 16).T.astype(np.int16)))
            ns = (num_idxs + P - 1) // P
            ops.append((c0 // P, ns, num_idxs, nval, buf, idx_cols_total, cols))
            idx_cols_total += cols

    idx_all = np.ascontiguousarray(
        np.tile(np.concatenate(idx_chunks, axis=1), (8, 1)))  # [128, cols]
    total_descs = sum(o[2] for o in ops)
    return dict(pi=pi, dinv_pad=dinv_pad, ops=ops, idx_all=idx_all,
                idx_cols_total=idx_cols_total, total_descs=total_descs,
                holes=holes, rounds=Kmax)


def build_constants(cfg: Cfg, prep, inputs):
    n, npad, nslot = cfg.n, cfg.npad, cfg.nslot
    pi = prep["pi"]
    dinv_pad = prep["dinv_pad"]

    pel_W = np.asarray(inputs["pel_W"], np.float32)
    pel_b = np.asarray(inputs["pel_b"], np.float32)
    pe_perm = (pel_W.T + pel_b)[pi]

    x = np.asarray(inputs["x"], np.float32)
    bs = x.shape[0]
    x_fm = np.zeros((bs, FIN, npad), np.float16)
    for s in range(bs):
        xc = np.concatenate([x[s][pi], pe_perm], axis=1)
        x_fm[s, :, :n] = xc.T.astype(np.float16)

    def to_node_major(v):
        return np.ascontiguousarray(v.reshape(nslot, P).T)

    dinv_nm = to_node_major(dinv_pad.astype(np.float16))
    dinv64 = np.ascontiguousarray(
        np.repeat(dinv_nm[:, :, None], HID, axis=2)).reshape(P, nslot * HID)
    mask = np.zeros(npad, np.float32)
    mask[:n] = 1.0
    mask_nm = to_node_major(mask)

    Wc = [np.ascontiguousarray(
        np.asarray(inputs[f"conv{i}_W"], np.float32).T.astype(np.float16))
        for i in (1, 2, 3)]
    bc = [np.ascontiguousarray(np.asarray(inputs[f"conv{i}_b"], np.float32)
                               .reshape(HID, 1)) for i in (1, 2, 3)]

    fc_W = np.asarray(inputs["fc_W"], np.float32).reshape(-1)
    w_l = [np.ascontiguousarray(fc_W[l::3].reshape(HID, 1).astype(np.float16))
           for l in range(3)]
    fc_b = float(np.asarray(inputs["fc_b"], np.float32).reshape(()))

    lin1_W = np.asarray(inputs["lin1_W"], np.float32)
    w1_perm = np.zeros((npad, NFC), np.float32)
    w1_perm[:n] = lin1_W[:, pi].T
    # node-major [P, nslot, NFC]: [p, g, :] = row of node g*128+p
    w1t_pg = np.ascontiguousarray(
        w1_perm.reshape(nslot, P, NFC).transpose(1, 0, 2)
        .astype(np.float16)).reshape(P, nslot * NFC)
    b1_eff = np.ascontiguousarray(
        (np.asarray(inputs["lin1_b"], np.float32)
         + fc_b * lin1_W.sum(axis=1)).reshape(1, NFC))

    return dict(x_fm=x_fm, dinv64=dinv64, mask_nm=mask_nm, Wc=Wc, bc=bc,
                w_l=w_l, w1t_pg=w1t_pg, b1_eff=b1_eff)


def build_program(cfg: Cfg, prep, gemm_grp=8, tr_grp=4, rb_grp=15):
    n, npad, nslot, sslot = cfg.n, cfg.npad, cfg.nslot, cfg.sslot
    ops = prep["ops"]
    f16 = mybir.dt.float16
    f32 = mybir.dt.float32

    nc = bacc.Bacc("TRN2", debug=False)

    x_dram = nc.dram_tensor("x_fm", [FIN, npad], f16, kind="ExternalInput")
    dinv_dram = nc.dram_tensor("dinv64", [P, nslot * HID], f16, kind="ExternalInput")
    mask_dram = nc.dram_tensor("mask_nm", [P, nslot], f32, kind="ExternalInput")
    Wc_dram = [nc.dram_tensor(f"Wc{i}", [FIN if i == 0 else HID, HID], f16,
                              kind="ExternalInput") for i in range(3)]
    bc_dram = [nc.dram_tensor(f"bc{i}", [HID, 1], f32, kind="ExternalInput")
               for i in range(3)]
    wl_dram = [nc.dram_tensor(f"wl{i}", [HID, 1], f16, kind="ExternalInput")
               for i in range(3)]
    idx_dram = nc.dram_tensor("idx_all", [P, prep["idx_cols_total"]],
                              mybir.dt.int16, kind="ExternalInput")
    w1t_dram = nc.dram_tensor("W1T", [P, nslot * NFC], f16, kind="ExternalInput")
    b1_dram = nc.dram_tensor("b1_eff", [1, NFC], f32, kind="ExternalInput")
    ident_dram = nc.dram_tensor("ident", [P, P], f16, kind="ExternalInput")
    z_dram = nc.dram_tensor("z", [1, NFC], f32, kind="ExternalOutput")

    s_hbm = nc.dram_tensor("s_hbm", [cfg.srows, 2 * HID], f16)

    with tile.TileContext(nc) as tc:
        with (
            tc.tile_pool(name="const", bufs=1) as cpool,
            tc.tile_pool(name="state", bufs=1) as spool,
            tc.tile_pool(name="sfull", bufs=2) as sfpool,
            tc.tile_pool(name="psum_t", bufs=2, space="PSUM") as pt_pool,
            tc.tile_pool(name="psum_tr", bufs=2, space="PSUM") as ptr_pool,
            tc.tile_pool(name="psum_g", bufs=1, space="PSUM") as pg_pool,
            tc.tile_pool(name="psum_z", bufs=1, space="PSUM") as pz_pool,
        ):
            dinv64 = cpool.tile([P, nslot, HID], f16, tag="dinv64")
            nc.sync.dma_start(out=dinv64[:], in_=dinv_dram[:].rearrange(
                "p (g f) -> p g f", f=HID))
            mask_sb = cpool.tile([P, nslot], f32, tag="mask")
            nc.sync.dma_start(out=mask_sb[:], in_=mask_dram[:])
            ident = cpool.tile([P, P], f16, tag="ident")
            nc.sync.dma_start(out=ident[:], in_=ident_dram[:])
            idx_sb = cpool.tile([P, prep["idx_cols_total"]], mybir.dt.int16,
                                tag="idx")
            nc.sync.dma_start(out=idx_sb[:], in_=idx_dram[:])
            Wc_sb, bc_sb, wl_sb = [], [], []
            for i in range(3):
                w = cpool.tile([FIN if i == 0 else HID, HID], f16, tag=f"Wc{i}")
                nc.sync.dma_start(out=w[:], in_=Wc_dram[i][:])
                Wc_sb.append(w)
                b = cpool.tile([HID, 1], f32, tag=f"bc{i}")
                nc.sync.dma_start(out=b[:], in_=bc_dram[i][:])
                bc_sb.append(b)
                wl = cpool.tile([HID, 1], f16, tag=f"wl{i}")
                nc.sync.dma_start(out=wl[:], in_=wl_dram[i][:])
                wl_sb.append(wl)
            b1_sb = cpool.tile([1, NFC], f32, tag="b1")
            nc.sync.dma_start(out=b1_sb[:], in_=b1_dram[:])
            # lin1 weights resident; load on the Activation HWDGE queue so the
            # big transfer doesn't head-block the SP queue
            w1t_sb = cpool.tile([P, nslot, NFC], f16, tag="w1t")
            nc.scalar.dma_start(out=w1t_sb[:], in_=w1t_dram[:].rearrange(
                "p (g f) -> p g f", f=NFC))

            m_sb = spool.tile([P, nslot, HID], f16, tag="m")
            z0_sb = spool.tile([P, rb_grp, HID], f16, tag="z0")
            nc.vector.memset(z0_sb[:], 0.0)
            s_nm = spool.tile([P, nslot, HID], f16, tag="s_nm")
            h_fm = spool.tile([HID, npad], f16, tag="h_fm")
            g16 = spool.tile([P, nslot], f16, tag="g16")
            # layer-0 GEMM input x lives in h_fm[0:FIN] until the first
            # post-aggregation write of h (strict data-flow ordering)
            nc.sync.dma_start(out=h_fm[0:FIN, :], in_=x_dram[:])

            # s_hbm views: rows (p, g) = p*sslot + g; node g*128+p at row slot
            # g < nslot; slot nslot-.. = per-partition dummy row
            s_pg = s_hbm[:].rearrange("(p g) f -> p g f", g=sslot)
            s_colA = s_hbm[:][:, 0:HID]
            s_colB = s_hbm[:][:, HID:2 * HID]

            psum_g = pg_pool.tile([P, nslot], f32, tag="pg")
            psum_z = pz_pool.tile([1, NFC], f32, tag="pz")

            for l in range(3):
                # GEMM -> m (fp16, node-major) -> init s rows (A=m self term)
                for g0 in range(0, nslot, gemm_grp):
                    gn = min(gemm_grp, nslot - g0)
                    psum_t = pt_pool.tile([P, gemm_grp, HID], f32, tag="pt")
                    src_fm = h_fm[0:FIN, :] if l == 0 else h_fm[:]
                    for j in range(gn):
                        nc.tensor.matmul(psum_t[:, j, :],
                                         src_fm[:, (g0 + j) * P:(g0 + j + 1) * P],
                                         Wc_sb[l][:], start=True, stop=True)
                    nc.vector.tensor_mul(m_sb[:, g0:g0 + gn, :],
                                         psum_t[:, :gn, :],
                                         dinv64[:, g0:g0 + gn, :])
                    nc.sync.dma_start(out=s_pg[:, g0:g0 + gn, 0:HID],
                                      in_=m_sb[:, g0:g0 + gn, :])
                # zero the B halves
                for z0s in range(0, nslot, rb_grp):
                    zn = min(rb_grp, nslot - z0s)
                    nc.sync.dma_start(out=s_pg[:, z0s:z0s + zn, HID:2 * HID],
                                      in_=z0_sb[:, :zn, :])
                # scatter rounds: s[dst] += m[src]
                for (c0s, ns, num_idxs, nval, buf, ioff, icols) in ops:
                    nc.gpsimd.dma_scatter_add(
                        s_colA if buf == 0 else s_colB,
                        m_sb[:, c0s:c0s + ns, :],
                        idx_sb[:, ioff:ioff + icols],
                        num_idxs, nval, HID, elem_step=2 * HID,
                        single_packet=False)
                # readback + post, chunked so PE/ACT overlap the readback DMA
                for r0 in range(0, nslot, rb_grp):
                    rn = min(rb_grp, nslot - r0)
                    sf = sfpool.tile([P, rb_grp, 2 * HID], f16, tag="sf")
                    nc.sync.dma_start(out=sf[:, :rn, :], in_=s_pg[:, r0:r0 + rn, :])
                    nc.vector.tensor_add(s_nm[:, r0:r0 + rn, :],
                                         sf[:, :rn, 0:HID],
                                         sf[:, :rn, HID:2 * HID])
                    nc.vector.tensor_mul(s_nm[:, r0:r0 + rn, :],
                                         s_nm[:, r0:r0 + rn, :],
                                         dinv64[:, r0:r0 + rn, :])
                    for t0 in range(r0, r0 + rn, tr_grp):
                        tn = min(tr_grp, r0 + rn - t0)
                        psum_tr = ptr_pool.tile([HID, tr_grp, P], f16, tag="ptr")
                        for j in range(tn):
                            nc.tensor.transpose(psum_tr[:, j, :],
                                                s_nm[:, t0 + j, :], ident[:])
                        nc.scalar.activation(
                            h_fm[:, t0 * P:(t0 + tn) * P],
                            psum_tr[:, :tn, :].rearrange("f g p -> f (g p)"),
                            mybir.ActivationFunctionType.Relu,
                            bias=bc_sb[l][:], scale=1.0)
                    for j in range(r0, r0 + rn):
                        nc.tensor.matmul(psum_g[:, j:j + 1],
                                         h_fm[:, j * P:(j + 1) * P],
                                         wl_sb[l][:], start=(l == 0), stop=(l == 2))

            # head: z = relu(g^T @ W1T + b1), g masked to real nodes
            nc.vector.tensor_mul(g16[:], psum_g[:], mask_sb[:])
            for j in range(nslot):
                nc.tensor.matmul(psum_z[:], g16[:, j:j + 1], w1t_sb[:, j, :],
                                 start=(j == 0), stop=(j == nslot - 1))
            z_sb = spool.tile([1, NFC], f32, tag="z")
            nc.vector.tensor_add(z_sb[:], psum_z[:], b1_sb[:])
            nc.vector.tensor_relu(z_sb[:], z_sb[:])
            nc.sync.dma_start(out=z_dram[:], in_=z_sb[:])

    nc.compile()
    return nc


def make_in_maps(cfg: Cfg, prep, consts, n_cores=N_CORES):
    eye = np.eye(P, dtype=np.float16)
    shared = dict(
        dinv64=consts["dinv64"], mask_nm=consts["mask_nm"],
        idx_all=prep["idx_all"], W1T=consts["w1t_pg"],
        b1_eff=consts["b1_eff"], ident=eye,
    )
    for i in range(3):
        shared[f"Wc{i}"] = consts["Wc"][i]
        shared[f"bc{i}"] = consts["bc"][i]
        shared[f"wl{i}"] = consts["w_l"][i]
    return [dict(shared, x_fm=np.ascontiguousarray(consts["x_fm"][c]))
            for c in range(n_cores)]


def finish_host(z_all, inputs):
    W2 = np.asarray(inputs["lin2_W"], np.float32)
    b2 = np.asarray(inputs["lin2_b"], np.float32)
    logits = z_all @ W2.T + b2
    mx = logits.max(axis=1, keepdims=True)
    e = np.exp(logits - mx)
    return ((logits - mx) - np.log(e.sum(axis=1, keepdims=True))).astype(np.float32)


_PROGRAM_CACHE = {}


def _get_program(cfg: Cfg, prep, cache_key):
    hit = _PROGRAM_CACHE.get(cache_key)
    if hit is None:
        hit = build_program(cfg, prep)
        _PROGRAM_CACHE[cache_key] = hit
    return hit


def _reset_device():
    """Run a trivial program to clear a wedged exec unit (observed to help)."""
    try:
        nc = bacc.Bacc("TRN2", debug=False)
        a = nc.dram_tensor("a", [P, 64], mybir.dt.float32, kind="ExternalInput")
        b = nc.dram_tensor("b", [P, 64], mybir.dt.float32, kind="ExternalOutput")
        with tile.TileContext(nc) as tc:
            with tc.tile_pool(name="p", bufs=1) as pool:
                t = pool.tile([P, 64], mybir.dt.float32)
                nc.sync.dma_start(out=t[:], in_=a[:])
                nc.sync.dma_start(out=b[:], in_=t[:])
        nc.compile()
        run_bass_kernel_spmd(
            nc, [{"a": np.zeros((P, 64), np.float32)}] * N_CORES,
            list(range(N_CORES)))
    except Exception:
        pass


def kernel(**inputs) -> np.ndarray:
    x = np.asarray(inputs["x"])
    bs, n = x.shape[0], x.shape[1]
    assert bs == N_CORES, f"expected batch {N_CORES}, got {bs}"

    cfg = Cfg(n=n)
    edge_index = np.asarray(inputs["edge_index"])
    prep = preprocess(cfg, edge_index)
    cache_key = (n, edge_index.shape[1], hash(edge_index.tobytes()))
    nc = _get_program(cfg, prep, cache_key)
    consts = build_constants(cfg, prep, inputs)
    in_maps = make_in_maps(cfg, prep, consts)

    last_err = None
    for attempt in range(3):
        try:
            res = run_bass_kernel_spmd(nc, in_maps, list(range(N_CORES)))
            break
        except Exception as e:  # wedged device — reset and retry
            last_err = e
            _reset_device()
    else:
        raise last_err

    z_all = np.stack([res.results[c]["z"][0] for c in range(N_CORES)])
    return finish_host(z_all, inputs)
